# revision 28
# baseline (speedup 1.0000x reference)
"""Trainium2 Bass kernel for nn_Conv2dLocal (locally-connected 2d conv,
no weight sharing).

Strategy: shard the 32 output rows across 8 NeuronCores (4 rows each).
Within a core, the 4 output rows form two PE row-groups (oh{0,1} on array
rows 0-63, oh{2,3} on rows 64-127) that run concurrently; inside a group
the two oh rows map to the two PE column-halves (also concurrent). Each x
pixel [c=64, b=64] is loaded as the stationary operand and reused by up
to 6 matmuls; the per-pixel weight slices stream as the moving operand.

The kernel is DMA-bound (weights are use-once), so the layout minimizes
HBM bytes: weights are host-cast to fp8e3 (e3m4, x pre-scaled by 1/64 to
compensate the x64 weight scale; mixed fp16 lhsT x fp8 rhs matmul is
supported) halving the dominant stream to 4.72MB/core; x is ONE
128-partition fp16 load [128, 4*32*64] (partitions 0-63 hold input rows
r0-1..r0+2 for group 0, partitions 64-127 hold rows r0+1..r0+4 for group
1 - no duplicate 64-partition half-rate loads) = 2.1MB; all input DMAs
ride the SP HWDGE FIFO in stream order (x, bias, then 4 weight chunks,
all prefetchable since w_bufs=nchunks) while output DMAs ride the ACT
ring so drain waits never stall input streaming. Bias is folded in with
a K=2 matmul that opens each PSUM bank (start=True covers all 128
partitions); drains are 8 [128,512] fp32->fp16 DVE copies each followed
by a per-block output DMA.
"""

import numpy as np
import ml_dtypes

import concourse.mybir as mybir
import concourse.tile as tile
from concourse import bacc
from concourse.bass_utils import run_bass_kernel_spmd

B = 64
C = 64
O = 64
OW = 32
N_CORES = 8
R = 4          # output rows per core
XR = 4         # x rows held per partition-group
XCOLS = XR * 32 * B          # 8192
WCOLS = 32 * 2 * 3 * 3 * 64  # 36864
WSCALE = 64.0
F8 = mybir.dt.float8e3
F16 = mybir.dt.float16
F32 = mybir.dt.float32

_NC_CACHE = {}


def _mm_descs():
    """Per-pixel matmul descriptors: (wp, g, hl, ohlp, s0, s1, blk)."""
    out = []
    for wp in range(32):
        i_lo = 1 if wp == 0 else 0
        i_hi = 1 if wp == 31 else 2
        segs = []
        s = i_lo
        for i in range(i_lo, i_hi + 1):
            if (wp - 1 + i) // 8 != (wp - 1 + s) // 8:
                segs.append((s, i - 1))
                s = i
        segs.append((s, i_hi))
        for g in (0, 1):
            rows = range(0, 4) if g == 0 else range(2, 6)
            for hl in rows:
                for ohlp in (0, 1):
                    if not 0 <= hl - 2 * g - ohlp <= 2:
                        continue
                    for (s0, s1) in segs:
                        out.append((wp, g, hl, ohlp, s0, s1, (wp - 1 + s0) // 8))
    return out


CHUNK_PIXELS = [8, 8, 8, 8]


def build(n_iter=1, ps_bufs=8, chunk_pixels=None, unroll=1):
    """Build a NEFF executing n_iter * unroll convolutions.

    unroll > 1 places several conv bodies inside one For_i iteration:
    the all-engine For_i barrier only runs once per `unroll` convs, and
    Tile's dataflow deps pipeline adjacent bodies (body k+1's weight
    chunks prefetch during body k's tail, so the PE never goes idle or
    cold between convs).
    """
    nc = bacc.Bacc("TRN2", target_bir_lowering=False, debug=False,
                   num_devices=N_CORES)
    x_d = nc.dram_tensor("xp", [128, XCOLS], F16, kind="ExternalInput")
    w_d = nc.dram_tensor("wt", [128, WCOLS], F8, kind="ExternalInput")
    b_d = nc.dram_tensor("bias", [4, 2048], F16, kind="ExternalInput")
    e_d = nc.dram_tensor("ones", [4, 128], F16, kind="ExternalInput")
    o_d = nc.dram_tensor("out", [2, 128, 2048], F16, kind="ExternalOutput")

    if chunk_pixels is None:
        chunk_pixels = CHUNK_PIXELS
    assert sum(chunk_pixels) == 32
    nchunks = len(chunk_pixels)
    chunk_lo = [sum(chunk_pixels[:i]) for i in range(nchunks)]
    pix2chunk = {}
    for cb in range(nchunks):
        for wp in range(chunk_lo[cb], chunk_lo[cb] + chunk_pixels[cb]):
            pix2chunk[wp] = cb

    with tile.TileContext(nc) as tc:
        with (
            tc.tile_pool(name="xpool", bufs=1) as xpool,
            tc.tile_pool(name="cpool", bufs=1) as cpool,
            tc.tile_pool(name="opool", bufs=1) as opool,
            tc.tile_pool(name="wpool", bufs=nchunks + 2) as wpool,
            tc.tile_pool(name="pspool", bufs=ps_bufs, space="PSUM") as pspool,
        ):
            x_sb = xpool.tile([128, XCOLS], F16)
            nc.sync.dma_start(out=x_sb[:], in_=x_d[:, :])
            bias_sb = cpool.tile([128, 2048], F16, tag="bias")
            ones_sb = cpool.tile([128, 128], F16, tag="ones")
            for g in (0, 1):
                # tiny loads ride the ACT ring so their fixed latency never
                # stalls the big input stream on the SP ring
                nc.scalar.dma_start(out=bias_sb[64 * g : 64 * g + 2, :],
                                    in_=b_d[2 * g : 2 * g + 2, :])
                nc.scalar.dma_start(out=ones_sb[64 * g : 64 * g + 2, :],
                                    in_=e_d[2 * g : 2 * g + 2, :])
            out_sb = [
                opool.tile([128, 2048], F16, tag=f"out{g}", name=f"out_sb{g}")
                for g in (0, 1)
            ]

            descs = _mm_descs()
            n_per_bank = {}
            for d in descs:
                key = (d[1], d[6])
                n_per_bank[key] = n_per_bank.get(key, 0) + 1
            by_pixel = {}
            for d in descs:
                by_pixel.setdefault(d[0], []).append(d)
            # Matmul starts are pc-monotone (strict FIFO): a stalled mm
            # blocks every later mm from starting. Emit each pixel's mms
            # round-robin across the 4 PE quadrants (g, ohlp) so no
            # quadrant chain head-of-line blocks the others.
            for wp, lst in by_pixel.items():
                quads = {}
                for d in lst:
                    quads.setdefault((d[1], d[3]), []).append(d)
                rr = []
                qkeys = sorted(quads)
                while any(quads[q] for q in qkeys):
                    for q in qkeys:
                        if quads[q]:
                            rr.append(quads[q].pop(0))
                by_pixel[wp] = rr

            PIXCOLS = 2 * 3 * 3 * 64   # weight cols per pixel (1152)

            def body(warm_head=True, warm_tail=True):
                # open all 8 PSUM banks with bias matmuls up front: useful
                # PE work right after the loop barrier (covers the first
                # chunk's DMA latency and re-warms the HAM clock gate).
                pt = {}
                seen = {}
                for blk in range(4):
                    for g in (0, 1):
                        t = pspool.tile([128, 512], F32, name="ps")
                        pt[(g, blk)] = t
                        seen[(g, blk)] = 0
                        nc.tensor.matmul(
                            t[0:128, 0:512],
                            ones_sb[64 * g : 64 * g + 2, 0:128],
                            bias_sb[64 * g : 64 * g + 2,
                                    blk * 512 : blk * 512 + 512],
                            start=True, stop=False)

                if warm_head:
                    # bridge the genuinely-idle hole between the bias opens
                    # (~1.9us) and chunk 0's arrival+receipt (~3.4us) with
                    # zero-accumulates (K=1 all-zero stationary row -> adds
                    # exact zeros to bank (0,3), whose group stops last), so
                    # the HAM clock gate reaches 2.4GHz before the real
                    # matmuls begin.
                    for _ in range(3):
                        nc.tensor.matmul(
                            pt[(0, 3)][0:64, 0:512],
                            ones_sb[0:1, 64:128],
                            bias_sb[0:1, 0:512],
                            start=False, stop=False)

                chunk = [None] * nchunks
                for wp in range(32):
                    cb = pix2chunk[wp]
                    if chunk[cb] is None:
                        w0 = chunk_lo[cb] * PIXCOLS
                        w1 = w0 + chunk_pixels[cb] * PIXCOLS
                        t = wpool.tile([128, w1 - w0], F8, name="wt_t")
                        nc.sync.dma_start(out=t[:], in_=w_d[:, w0:w1])
                        chunk[cb] = t
                    for (_, g, hl, ohlp, s0, s1, blk) in by_pixel[wp]:
                        p0 = 64 * g
                        t = pt[(g, blk)]
                        seen[(g, blk)] += 1
                        last = seen[(g, blk)] == n_per_bank[(g, blk)]
                        kh = hl - 2 * g - ohlp
                        n = (s1 - s0 + 1) * 64
                        fo = ((wp - 1 + s0) % 8) * 64
                        base = (((wp * 2 + ohlp) * 3 + kh) * 3) * 64 \
                            - chunk_lo[cb] * PIXCOLS
                        nc.tensor.matmul(
                            t[64 * ohlp : 64 * ohlp + 64, fo : fo + n],
                            x_sb[p0 : p0 + 64,
                                 ((hl - 2 * g) * 32 + wp) * B
                                 : ((hl - 2 * g) * 32 + wp) * B + B],
                            chunk[cb][p0 : p0 + 64,
                                      base + s0 * 64 : base + s0 * 64 + n],
                            start=False, stop=last)
                    if wp >= 8 and (wp - 8) % 8 == 0:
                        # bank wp//8-1 is complete: drain + store while the
                        # weight stream continues (overlapped).
                        blk = wp // 8 - 1
                        for g in (0, 1):
                            nc.vector.tensor_copy(
                                out=out_sb[g][:, blk * 512 : blk * 512 + 512],
                                in_=pt[(g, blk)][:, :])
                            nc.scalar.dma_start(
                                out=o_d[g][:, blk * 512 : blk * 512 + 512],
                                in_=out_sb[g][:, blk * 512 : blk * 512 + 512])
                    if wp == 30:
                        # psum cols 0:384 of bank 3 (ow 24-29) see their last
                        # accumulate at pixel 30: drain + store them early so
                        # only a [128,128] slice remains after the final pixel.
                        for g in (0, 1):
                            nc.vector.tensor_copy(
                                out=out_sb[g][:, 1536:1920],
                                in_=pt[(g, 3)][:, 0:384])
                            nc.scalar.dma_start(
                                out=o_d[g][:, 1536:1920],
                                in_=out_sb[g][:, 1536:1920])
                for g in (0, 1):
                    nc.vector.tensor_copy(
                        out=out_sb[g][:, 1920:2048], in_=pt[(g, 3)][:, 384:512])
                for g in (0, 1):
                    nc.scalar.dma_start(
                        out=o_d[g][:, 1920:2048],
                        in_=out_sb[g][:, 1920:2048])
                if warm_tail:
                    # keep-alive: dummy matmuls into the already-drained
                    # bank (WAR on the drain copies orders them after the
                    # drain). They keep the PE busy through the final
                    # out-DMA receipt + barrier window so the HAM activity
                    # monitor never sees a full 3.4us idle window; the next
                    # iteration's start=True bank opens wipe the garbage.
                    for _ in range(8):
                        nc.tensor.matmul(
                            pt[(0, 3)][0:64, 0:512],
                            ones_sb[0:2, 0:64],
                            bias_sb[0:2, 0:512],
                            start=True, stop=True)

            def bodies():
                for u in range(unroll):
                    body(warm_head=(u == 0), warm_tail=(u == unroll - 1))

            if n_iter == 1:
                bodies()
            else:
                with tc.For_i(0, n_iter, 1,
                              hint_engines=(mybir.EngineType.PE,)):
                    bodies()
    nc.compile()
    return nc


def get_nc():
    if "nc" not in _NC_CACHE:
        _NC_CACHE["nc"] = build()
    return _NC_CACHE["nc"]


# ---------------- host-side layout prep ----------------

def prep_x(x):
    xt = (x.astype(np.float32) * (1.0 / WSCALE)).transpose(1, 2, 3, 0)
    # padded rows h' = -1..32 (34 rows); pixel columns w = 0..31 (no w pad:
    # boundary kw taps address clipped outputs, never out-of-range x).
    xp = np.zeros((C, 34, 32, B), np.float16)
    xp[:, 1:33] = xt
    outs = []
    for c in range(N_CORES):
        r0 = R * c
        halves = [
            xp[:, r0 + 2 * g : r0 + 2 * g + XR].reshape(C, XCOLS)
            for g in (0, 1)
        ]
        outs.append(np.ascontiguousarray(np.concatenate(halves, axis=0)))
    return outs


def prep_w(weight):
    w64 = weight.astype(np.float32) * WSCALE
    outs = []
    for core in range(N_CORES):
        r0 = R * core
        Wc = w64[r0 : r0 + 4]                          # [4, 32, O, C, KH, KW]
        T = Wc.transpose(0, 1, 4, 5, 3, 2)             # [ohl, ow, kh, kw, c, o]
        halves = []
        for g in (0, 1):
            wt_g = np.zeros((32, 2, 3, 3, C, O), np.float32)
            for i in (0, 1, 2):
                kw = 2 - i
                lo, hi = max(0, 1 - i), min(32, 33 - i)
                wt_g[lo:hi, :, :, i] = T[2 * g : 2 * g + 2,
                                         lo - 1 + i : hi - 1 + i, :, kw
                                         ].transpose(1, 0, 2, 3, 4)
            halves.append(
                wt_g.reshape(32 * 2 * 3 * 3, C, O).transpose(1, 0, 2)
                .reshape(C, WCOLS))
        outs.append(np.ascontiguousarray(
            np.concatenate(halves, axis=0)).astype(ml_dtypes.float8_e3m4))
    return outs


def prep_bias(bias):
    outs = []
    for core in range(N_CORES):
        bc = bias[:, R * core : R * core + 4, :]       # [O, 4, OW]
        rows = [np.ascontiguousarray(bc[:, r, :].T).reshape(2048)
                for r in range(4)]                     # [ow, o] flattened
        outs.append(np.stack(rows).astype(np.float16))
    return outs


def prep_ones():
    e = np.zeros((4, 128), np.float16)
    e[0, 0:64] = 1.0
    e[1, 64:128] = 1.0
    e[2, 0:64] = 1.0
    e[3, 64:128] = 1.0
    return e


def make_in_maps(x, weight, bias):
    xs = prep_x(np.asarray(x, np.float32))
    ws = prep_w(np.asarray(weight, np.float32))
    bs = prep_bias(np.asarray(bias, np.float32))
    e = prep_ones()
    return [{"xp": xs[c], "wt": ws[c], "bias": bs[c], "ones": e}
            for c in range(N_CORES)]


def assemble_out(per_core):
    out = np.empty((B, O, 32, OW), np.float32)
    for core in range(N_CORES):
        r0 = R * core
        dev = np.asarray(per_core[core], np.float32).reshape(2, 2, B, OW, O)
        for g in (0, 1):
            for ohlp in (0, 1):
                out[:, :, r0 + 2 * g + ohlp, :] = dev[g, ohlp].transpose(0, 2, 1)
    return out


def kernel(x, weight, bias):
    nc = get_nc()
    in_maps = make_in_maps(x, weight, bias)
    res = run_bass_kernel_spmd(nc, in_maps, core_ids=list(range(N_CORES)))
    return assemble_out([res.results[c]["out"] for c in range(N_CORES)])


# revision 35
# speedup vs baseline: 4.3899x; 4.3899x over previous
"""Trainium2 Bass kernel for nn_Conv2dLocal (locally-connected 2d conv,
no weight sharing).

Strategy: shard the 32 output rows across 8 NeuronCores (4 rows each).
Within a core, the 4 output rows form two PE row-groups (oh{0,1} on array
rows 0-63, oh{2,3} on rows 64-127) that run concurrently; inside a group
the two oh rows map to the two PE column-halves (also concurrent). Each x
pixel [c=64, b=64] is loaded as the stationary operand and reused by up
to 6 matmuls; the per-pixel weight slices stream as the moving operand.

The kernel is DMA-bound (weights are use-once), so the layout minimizes
HBM bytes: weights are host-cast to fp8e3 (e3m4, x pre-scaled by 1/64 to
compensate the x64 weight scale; mixed fp16 lhsT x fp8 rhs matmul is
supported) halving the dominant stream to 4.72MB/core; x is ONE
128-partition fp16 load [128, 4*32*64] (partitions 0-63 hold input rows
r0-1..r0+2 for group 0, partitions 64-127 hold rows r0+1..r0+4 for group
1 - no duplicate 64-partition half-rate loads) = 2.1MB; all input DMAs
ride the SP HWDGE FIFO in stream order (x, bias, then 4 weight chunks,
all prefetchable since w_bufs=nchunks) while output DMAs ride the ACT
ring so drain waits never stall input streaming. Bias is folded in with
a K=2 matmul that opens each PSUM bank (start=True covers all 128
partitions); drains are 8 [128,512] fp32->fp16 DVE copies each followed
by a per-block output DMA.
"""

import numpy as np
import ml_dtypes

import concourse.mybir as mybir
import concourse.tile as tile
from concourse import bacc
from concourse.bass_utils import run_bass_kernel_spmd

B = 64
C = 64
O = 64
OW = 32
N_CORES = 8
R = 4          # output rows per core
XR = 4         # x rows held per partition-group
XCOLS = XR * 32 * B          # 8192
WCOLS = 32 * 2 * 3 * 3 * 64  # 36864
WSCALE = 64.0
F8 = mybir.dt.float8e3
F16 = mybir.dt.float16
F32 = mybir.dt.float32

_NC_CACHE = {}


def _mm_descs():
    """Per-pixel matmul descriptors: (wp, g, hl, ohlp, s0, s1, blk)."""
    out = []
    for wp in range(32):
        i_lo = 1 if wp == 0 else 0
        i_hi = 1 if wp == 31 else 2
        segs = []
        s = i_lo
        for i in range(i_lo, i_hi + 1):
            if (wp - 1 + i) // 8 != (wp - 1 + s) // 8:
                segs.append((s, i - 1))
                s = i
        segs.append((s, i_hi))
        for g in (0, 1):
            rows = range(0, 4) if g == 0 else range(2, 6)
            for hl in rows:
                for ohlp in (0, 1):
                    if not 0 <= hl - 2 * g - ohlp <= 2:
                        continue
                    for (s0, s1) in segs:
                        out.append((wp, g, hl, ohlp, s0, s1, (wp - 1 + s0) // 8))
    return out


CHUNK_PIXELS = [8, 8, 8, 8]


def build(n_iter=1, ps_bufs=8, chunk_pixels=None, unroll=1):
    """Build a NEFF executing n_iter * unroll convolutions.

    unroll > 1 places several conv bodies inside one For_i iteration:
    the all-engine For_i barrier only runs once per `unroll` convs, and
    Tile's dataflow deps pipeline adjacent bodies (body k+1's weight
    chunks prefetch during body k's tail, so the PE never goes idle or
    cold between convs).
    """
    nc = bacc.Bacc("TRN2", target_bir_lowering=False, debug=False,
                   num_devices=N_CORES)
    x_d = nc.dram_tensor("xp", [128, XCOLS], F16, kind="ExternalInput")
    w_d = nc.dram_tensor("wt", [128, WCOLS], F8, kind="ExternalInput")
    b_d = nc.dram_tensor("biasb", [2, 128, 2048], F16, kind="ExternalInput")
    o_d = nc.dram_tensor("out", [2, 128, 2048], F16, kind="ExternalOutput")

    if chunk_pixels is None:
        chunk_pixels = CHUNK_PIXELS
    assert sum(chunk_pixels) == 32
    nchunks = len(chunk_pixels)
    chunk_lo = [sum(chunk_pixels[:i]) for i in range(nchunks)]
    pix2chunk = {}
    for cb in range(nchunks):
        for wp in range(chunk_lo[cb], chunk_lo[cb] + chunk_pixels[cb]):
            pix2chunk[wp] = cb

    with tile.TileContext(nc) as tc:
        with (
            tc.tile_pool(name="xpool", bufs=1) as xpool,
            tc.tile_pool(name="cpool", bufs=1) as cpool,
            tc.tile_pool(name="opool", bufs=1) as opool,
            tc.tile_pool(name="wpool", bufs=nchunks + 2) as wpool,
            tc.tile_pool(name="pspool", bufs=ps_bufs, space="PSUM") as pspool,
        ):
            x_sb = xpool.tile([128, XCOLS], F16)
            nc.sync.dma_start(out=x_sb[:], in_=x_d[:, :])
            # bias pre-broadcast across the 64 batch partitions per
            # (g, ohlp) half; added during the DVE drains (not via PE
            # matmuls - those cost ~5us/conv of pure PE issue time).
            bias_bc = [
                cpool.tile([128, 2048], F16, tag=f"biasb{g}",
                           name=f"bias_bc{g}")
                for g in (0, 1)
            ]
            for g in (0, 1):
                # loads ride the ACT ring so their latency never stalls
                # the weight stream on the SP ring (loaded once, resident)
                nc.scalar.dma_start(out=bias_bc[g][:], in_=b_d[g][:, :])
            out_sb = [
                opool.tile([128, 2048], F16, tag=f"out{g}", name=f"out_sb{g}")
                for g in (0, 1)
            ]

            descs = _mm_descs()
            n_per_bank = {}
            for d in descs:
                key = (d[1], d[6])
                n_per_bank[key] = n_per_bank.get(key, 0) + 1
            by_pixel = {}
            for d in descs:
                by_pixel.setdefault(d[0], []).append(d)
            # Matmul starts are pc-monotone (strict FIFO): a stalled mm
            # blocks every later mm from starting. Emit each pixel's mms
            # round-robin across the 4 PE quadrants (g, ohlp) so no
            # quadrant chain head-of-line blocks the others.
            for wp, lst in by_pixel.items():
                quads = {}
                for d in lst:
                    quads.setdefault((d[1], d[3]), []).append(d)
                rr = []
                qkeys = sorted(quads)
                while any(quads[q] for q in qkeys):
                    for q in qkeys:
                        if quads[q]:
                            rr.append(quads[q].pop(0))
                by_pixel[wp] = rr

            PIXCOLS = 2 * 3 * 3 * 64   # weight cols per pixel (1152)

            def body(warm_head=True, warm_tail=True):
                pt = {}
                seen = {}
                opened = set()
                for blk in range(4):
                    for g in (0, 1):
                        pt[(g, blk)] = pspool.tile([128, 512], F32, name="ps")
                        seen[(g, blk)] = 0

                if warm_head:
                    # bridge the idle hole until chunk 0's arrival+receipt
                    # with dummy matmuls into bank (0,3) (values irrelevant:
                    # its first real start=True matmul wipes the bank), so
                    # the HAM clock gate reaches 2.4GHz before the real
                    # matmuls begin.
                    for _ in range(4):
                        nc.tensor.matmul(
                            pt[(0, 3)][0:64, 0:512],
                            bias_bc[0][0:1, 64:128],
                            bias_bc[0][0:1, 0:512],
                            start=True, stop=True)

                chunk = [None] * nchunks
                for wp in range(32):
                    cb = pix2chunk[wp]
                    if chunk[cb] is None:
                        w0 = chunk_lo[cb] * PIXCOLS
                        w1 = w0 + chunk_pixels[cb] * PIXCOLS
                        t = wpool.tile([128, w1 - w0], F8, name="wt_t")
                        nc.sync.dma_start(out=t[:], in_=w_d[:, w0:w1])
                        chunk[cb] = t
                    for (_, g, hl, ohlp, s0, s1, blk) in by_pixel[wp]:
                        p0 = 64 * g
                        t = pt[(g, blk)]
                        seen[(g, blk)] += 1
                        last = seen[(g, blk)] == n_per_bank[(g, blk)]
                        # first matmul touching this bank's partition half
                        # opens it: start=True clears has_written for the
                        # written partitions, so each element's first writer
                        # overwrites and later writers accumulate.
                        first = (g, blk, ohlp) not in opened
                        opened.add((g, blk, ohlp))
                        kh = hl - 2 * g - ohlp
                        n = (s1 - s0 + 1) * 64
                        fo = ((wp - 1 + s0) % 8) * 64
                        base = (((wp * 2 + ohlp) * 3 + kh) * 3) * 64 \
                            - chunk_lo[cb] * PIXCOLS
                        nc.tensor.matmul(
                            t[64 * ohlp : 64 * ohlp + 64, fo : fo + n],
                            x_sb[p0 : p0 + 64,
                                 ((hl - 2 * g) * 32 + wp) * B
                                 : ((hl - 2 * g) * 32 + wp) * B + B],
                            chunk[cb][p0 : p0 + 64,
                                      base + s0 * 64 : base + s0 * 64 + n],
                            start=first, stop=last)
                    if wp >= 8 and (wp - 8) % 8 == 0:
                        # bank wp//8-1 is complete: drain (+bias add) +
                        # store while the weight stream continues.
                        blk = wp // 8 - 1
                        for g in (0, 1):
                            nc.vector.tensor_add(
                                out=out_sb[g][:, blk * 512 : blk * 512 + 512],
                                in0=pt[(g, blk)][:, :],
                                in1=bias_bc[g][:, blk * 512 : blk * 512 + 512])
                            nc.scalar.dma_start(
                                out=o_d[g][:, blk * 512 : blk * 512 + 512],
                                in_=out_sb[g][:, blk * 512 : blk * 512 + 512])
                    if wp == 30:
                        # psum cols 0:384 of bank 3 (ow 24-29) see their last
                        # accumulate at pixel 30: drain + store them early so
                        # only a [128,128] slice remains after the final pixel.
                        for g in (0, 1):
                            nc.vector.tensor_add(
                                out=out_sb[g][:, 1536:1920],
                                in0=pt[(g, 3)][:, 0:384],
                                in1=bias_bc[g][:, 1536:1920])
                            nc.scalar.dma_start(
                                out=o_d[g][:, 1536:1920],
                                in_=out_sb[g][:, 1536:1920])
                for g in (0, 1):
                    nc.vector.tensor_add(
                        out=out_sb[g][:, 1920:2048],
                        in0=pt[(g, 3)][:, 384:512],
                        in1=bias_bc[g][:, 1920:2048])
                for g in (0, 1):
                    nc.scalar.dma_start(
                        out=o_d[g][:, 1920:2048],
                        in_=out_sb[g][:, 1920:2048])
                if warm_tail:
                    # keep-alive: dummy matmuls into the already-drained
                    # bank (WAR on the drain copies orders them after the
                    # drain). They keep the PE busy through the final
                    # out-DMA receipt + barrier window so the HAM activity
                    # monitor never sees a full 3.4us idle window; the next
                    # iteration's bank opens wipe the garbage.
                    for _ in range(8):
                        nc.tensor.matmul(
                            pt[(0, 3)][0:64, 0:512],
                            bias_bc[0][0:2, 0:64],
                            bias_bc[0][0:2, 0:512],
                            start=True, stop=True)

            def bodies():
                for u in range(unroll):
                    body(warm_head=(u == 0), warm_tail=(u == unroll - 1))

            if n_iter == 1:
                bodies()
            else:
                with tc.For_i(0, n_iter, 1,
                              hint_engines=(mybir.EngineType.PE,)):
                    bodies()
    nc.compile()
    return nc


def get_nc():
    if "nc" not in _NC_CACHE:
        _NC_CACHE["nc"] = build()
    return _NC_CACHE["nc"]


# ---------------- host-side layout prep ----------------

def prep_x(x):
    xt = (x.astype(np.float32) * (1.0 / WSCALE)).transpose(1, 2, 3, 0)
    # padded rows h' = -1..32 (34 rows); pixel columns w = 0..31 (no w pad:
    # boundary kw taps address clipped outputs, never out-of-range x).
    xp = np.zeros((C, 34, 32, B), np.float16)
    xp[:, 1:33] = xt
    outs = []
    for c in range(N_CORES):
        r0 = R * c
        halves = [
            xp[:, r0 + 2 * g : r0 + 2 * g + XR].reshape(C, XCOLS)
            for g in (0, 1)
        ]
        outs.append(np.ascontiguousarray(np.concatenate(halves, axis=0)))
    return outs


def prep_w(weight):
    w64 = weight.astype(np.float32) * WSCALE
    outs = []
    for core in range(N_CORES):
        r0 = R * core
        Wc = w64[r0 : r0 + 4]                          # [4, 32, O, C, KH, KW]
        T = Wc.transpose(0, 1, 4, 5, 3, 2)             # [ohl, ow, kh, kw, c, o]
        halves = []
        for g in (0, 1):
            wt_g = np.zeros((32, 2, 3, 3, C, O), np.float32)
            for i in (0, 1, 2):
                kw = 2 - i
                lo, hi = max(0, 1 - i), min(32, 33 - i)
                wt_g[lo:hi, :, :, i] = T[2 * g : 2 * g + 2,
                                         lo - 1 + i : hi - 1 + i, :, kw
                                         ].transpose(1, 0, 2, 3, 4)
            halves.append(
                wt_g.reshape(32 * 2 * 3 * 3, C, O).transpose(1, 0, 2)
                .reshape(C, WCOLS))
        outs.append(np.ascontiguousarray(
            np.concatenate(halves, axis=0)).astype(ml_dtypes.float8_e3m4))
    return outs


def prep_bias(bias):
    """Per core: [2(g), 128(ohlp,b), 2048(ow,o)] with the per-(g,ohlp) bias
    row broadcast across the 64 batch partitions."""
    outs = []
    for core in range(N_CORES):
        bc = bias[:, R * core : R * core + 4, :]       # [O, 4, OW]
        rows = [np.ascontiguousarray(bc[:, r, :].T).reshape(2048)
                for r in range(4)]                     # [ow, o] flattened
        per_g = []
        for g in (0, 1):
            halves = [np.broadcast_to(rows[2 * g + ohlp], (64, 2048))
                      for ohlp in (0, 1)]
            per_g.append(np.concatenate(halves, axis=0))
        outs.append(np.ascontiguousarray(np.stack(per_g)).astype(np.float16))
    return outs


def make_in_maps(x, weight, bias):
    xs = prep_x(np.asarray(x, np.float32))
    ws = prep_w(np.asarray(weight, np.float32))
    bs = prep_bias(np.asarray(bias, np.float32))
    return [{"xp": xs[c], "wt": ws[c], "biasb": bs[c]}
            for c in range(N_CORES)]


def assemble_out(per_core):
    out = np.empty((B, O, 32, OW), np.float32)
    for core in range(N_CORES):
        r0 = R * core
        dev = np.asarray(per_core[core], np.float32).reshape(2, 2, B, OW, O)
        for g in (0, 1):
            for ohlp in (0, 1):
                out[:, :, r0 + 2 * g + ohlp, :] = dev[g, ohlp].transpose(0, 2, 1)
    return out


def kernel(x, weight, bias):
    nc = get_nc()
    in_maps = make_in_maps(x, weight, bias)
    res = run_bass_kernel_spmd(nc, in_maps, core_ids=list(range(N_CORES)))
    return assemble_out([res.results[c]["out"] for c in range(N_CORES)])
